# revision 29
# baseline (speedup 1.0000x reference)
"""Trainium2 Bass kernel for a pre-LN multi-head self-attention block.

Problem: y = out_proj(MHA(LayerNorm(x))) with B=8, N=1024, E=768, H=12.

Sharding: pure data-parallel — batch element b runs on core b (8 cores, no
collectives). Host-side prep: transposes, fp16 weight conversion, and folding
the LN affine into the QKV projection (w_qkv·diag(g), b_qkv + W@ln_b), so the
on-device LN is just (x - mu)·rstd.

Per-core design (feature-major; no PE transposes anywhere):
  1. LayerNorm stats via ones-vector matmuls (sum / sum-of-squares),
     rstd = exp(-0.5*ln(var+eps)) on ACT, normalize = 2 DVE ops per chunk.
  2. QKV projection in fp16. Q^T/K^T feature-major [f, tok] with a head PAIR
     packed per 128-partition tile (h0 in partitions 0-63, h1 in 64-127);
     V token-major with a ones column per 65-wide head slab (PV's extra
     output row accumulates softmax denominators for free). Q pre-scaled.
  3. Attention processed in HEAD PAIRS: the two heads' score matmuls
     (contraction = head_dim = 64) are emitted back-to-back as 64x128-mode
     row tiles (tile_position (0,0) and (64,0)) so they run CONCURRENTLY in
     the PE array — 2x score throughput vs sequential heads. exp(S^T - 9) on
     ACT per head; ACT is the near-critical engine so everything else hides
     under it.
  4. PV for the previous pair is interleaved into the current pair's steps
     (h0 during kt 0-3, h1 during kt 4-7; ctx PSUM recycled in between).
     Per-head normalize right after evacuation: reciprocal_approx_fast on the
     denominator row, gpsimd broadcast, one DVE multiply.
  5. V projection and the remaining QKV ftiles run as PE "fillers" during the
     ACT-bound attention steps; out-projection epochs are appended as their
     CT chunks complete. Keeps the PE dense (HAM stays at 2.4 GHz).
"""

import sys

sys.path.insert(0, "/opt/trn_rl_repo")

import numpy as np

import concourse.bass as bass
import concourse.tile as tile
from concourse import bacc, mybir
from concourse import bass_utils

F32 = mybir.dt.float32
F32R = mybir.dt.float32r
F16 = mybir.dt.float16
ALU = mybir.AluOpType
ACTF = mybir.ActivationFunctionType

B, N, E, H, HD = 8, 1024, 768, 12, 64
F3 = 3 * E  # 2304
EC = E // 128  # 6 feature chunks
TT = N // 128  # 8 token tiles
NP = H // 2  # 6 head pairs
EPS = 1e-5

_cache = {}


def _build_kernel():
    nc = bacc.Bacc(
        "TRN2", target_bir_lowering=False, debug=False, num_devices=B
    )

    xT_d = nc.dram_tensor("xT", [E, N], F16, kind="ExternalInput").ap()
    wq_d = nc.dram_tensor("wqkvT", [E, F3], F16, kind="ExternalInput").ap()
    wo_d = nc.dram_tensor("woutT", [E, E], F16, kind="ExternalInput").ap()
    bqk_d = nc.dram_tensor("bqk", [128, 12], F32, kind="ExternalInput").ap()
    bv_d = nc.dram_tensor("bv_b", [128, E], F32, kind="ExternalInput").ap()
    bo_d = nc.dram_tensor("bo_b", [128, E], F32, kind="ExternalInput").ap()
    out_d = nc.dram_tensor("out", [N, E], F32, kind="ExternalOutput").ap()

    with tile.TileContext(nc) as tc:
        _emit(nc, tc, xT_d, wq_d, wo_d, bqk_d, bv_d, bo_d, out_d)

    nc.compile()
    return nc


def _emit(nc, tc, xT_d, wq_d, wo_d, bqk_d, bv_d, bo_d, out_d):
    from contextlib import ExitStack

    with ExitStack() as octx:
        # ---- long-lived pools ----
        cpool = octx.enter_context(tc.tile_pool(name="consts", bufs=1))
        qt_pool = octx.enter_context(tc.tile_pool(name="qt", bufs=1))
        kt_pool = octx.enter_context(tc.tile_pool(name="kt", bufs=1))
        v_pool = octx.enter_context(tc.tile_pool(name="v", bufs=1))
        xn_pool = octx.enter_context(tc.tile_pool(name="xn", bufs=1))
        wq_pool = octx.enter_context(tc.tile_pool(name="wq", bufs=1))
        wo_pool = octx.enter_context(tc.tile_pool(name="wo", bufs=1))
        ct_pool = octx.enter_context(tc.tile_pool(name="ctxT", bufs=1))

        # consts + w_out ride the gpsimd SWDGE queue; x keeps the sync queue
        # and w_qkv the scalar HWDGE queue so the three input streams overlap.
        bqk = cpool.tile([128, 12], F32)
        nc.gpsimd.dma_start(bqk[:], bqk_d[:])
        bv = cpool.tile([128, E], F32)
        nc.gpsimd.dma_start(bv[:], bv_d[:])
        bo = cpool.tile([128, E], F32)
        nc.gpsimd.dma_start(bo[:], bo_d[:])
        ones_col = cpool.tile([128, 1], F16)
        nc.vector.tensor_copy(ones_col[:], nc.const_aps.tensor(1.0, (128, 1)))
        # constant shift inside exp: keeps unnormalized P within fp16 range
        # (softmax is invariant to it; denominators scale uniformly)
        negc = cpool.tile([128, 1], F32)
        nc.vector.memset(negc[:], -9.0)

        QT = [qt_pool.tile([128, N], F16, tag=f"qt{i}", name=f"qt{i}") for i in range(EC)]
        KT = [kt_pool.tile([128, N], F16, tag=f"kt{i}", name=f"kt{i}") for i in range(EC)]
        VW = 65 * H  # 780: 64 features + ones column per head
        V = [v_pool.tile([128, VW], F16, tag=f"v{i}", name=f"v{i}") for i in range(TT)]
        XN = [xn_pool.tile([128, N], F16, tag=f"xn{i}", name=f"xn{i}") for i in range(EC)]
        CT = [ct_pool.tile([128, N], F16, tag=f"ct{i}", name=f"ct{i}") for i in range(EC)]
        wq = [wq_pool.tile([128, F3], F16, tag=f"w{i}", name=f"w{i}") for i in range(EC)]
        wo = [wo_pool.tile([128, E], F16, tag=f"wo{i}", name=f"wo{i}") for i in range(EC)]

        # ================= phase 1: load x, LN =================
        with (
            tc.tile_pool(name="xt", bufs=1) as xt_pool,
            tc.tile_pool(name="tmp", bufs=2) as tmp_pool,
            tc.tile_pool(name="rows", bufs=3) as row_pool,
            tc.tile_pool(name="bcast", bufs=1) as bc_pool,
        ):
            xt = [xt_pool.tile([128, N], F16, tag=f"x{i}", name=f"x{i}") for i in range(EC)]
            for i in range(EC):
                nc.sync.dma_start(xt[i][:], xT_d[i * 128 : (i + 1) * 128, :])
            # w_qkv sliced by column groups, highest-priority first: the
            # pre-loop ftiles (Q0,Q1 = cols 0:256; K0,K1 = 768:1024) now;
            # V + remaining Q/K slices are emitted after the LN row chain so
            # their queue-issue cost doesn't delay the Ln/Exp activations.
            for lo, hi in ((0, 256), (768, 1024)):
                for i in range(EC):
                    nc.scalar.dma_start(
                        wq[i][:, lo:hi], wq_d[i * 128 : (i + 1) * 128, lo:hi]
                    )

            with tc.tile_pool(name="stats_ps", bufs=1, space="PSUM") as stats_ps:
                ps_sum = stats_ps.tile([1, N], F32)
                ps_sq = stats_ps.tile([1, N], F32)
                for i in range(EC):
                    xsq = tmp_pool.tile([128, N], F16, tag="tmp", name="xsq")
                    nc.vector.tensor_tensor(
                        xsq[:], xt[i][:], xt[i][:], ALU.mult,
                    )
                    st, sp = i == 0, i == EC - 1
                    for hf in range(2):
                        sl = slice(hf * 512, hf * 512 + 512)
                        nc.tensor.matmul(
                            ps_sum[:, sl], ones_col[:], xt[i][:, sl],
                            start=st, stop=sp,
                        )
                        nc.tensor.matmul(
                            ps_sq[:, sl], ones_col[:], xsq[:, sl],
                            start=st, stop=sp,
                        )
                    # density keepers on the just-arrived chunk: the stats
                    # stream alone is too sparse for HAM to unthrottle
                    wscr = stats_ps.tile([1, 512], F32, tag="wscr")
                    for w in range(2):
                        nc.tensor.matmul(
                            wscr[:], ones_col[:], xt[i][:, 0:512],
                            start=True, stop=True,
                        )
                # warmth keepers: harmless matmuls that bridge the PE-idle
                # window while mu/rstd/normalize run, so HAM stays at 2.4GHz
                # when the QKV projection starts
                wscr = stats_ps.tile([1, 512], F32, tag="wscr")
                for w in range(24):
                    nc.tensor.matmul(
                        wscr[:], ones_col[:], xt[w % EC][:, 0:512],
                        start=True, stop=True,
                    )

                mu_row = row_pool.tile([1, N], F32, tag="row", name="mu_row")
                nc.vector.tensor_scalar_mul(mu_row[:], ps_sum[:], 1.0 / E)
                msq_row = row_pool.tile([1, N], F32, tag="row", name="msq_row")
                nc.vector.tensor_tensor(msq_row[:], mu_row[:], mu_row[:], ALU.mult)
                var_row = row_pool.tile([1, N], F32, tag="row", name="var_row")
                nc.vector.scalar_tensor_tensor(
                    var_row[:], ps_sq[:], 1.0 / E, msq_row[:],
                    ALU.mult, ALU.subtract,
                )
            eps_ap = row_pool.tile([1, 1], F32)
            nc.vector.memset(eps_ap[:], EPS)
            # rstd = exp(-0.5 * ln(var + eps)) — both on ACT (fast row ops)
            lnv_row = row_pool.tile([1, N], F32, tag="row", name="lnv_row")
            nc.scalar.activation(lnv_row[:], var_row[:], ACTF.Ln, bias=eps_ap[:])
            rstd_row = row_pool.tile([1, N], F32, tag="row", name="rstd_row")
            nc.scalar.activation(rstd_row[:], lnv_row[:], ACTF.Exp, scale=-0.5)

            # remaining w_qkv slices: V first (pair-0 fillers need it);
            # w_out rides the gpsimd queue here — it isn't needed until the
            # first out-proj epoch, so keep it off the startup-critical BW
            for lo, hi in ((1536, 2304), (256, 768), (1024, 1536)):
                for i in range(EC):
                    nc.scalar.dma_start(
                        wq[i][:, lo:hi], wq_d[i * 128 : (i + 1) * 128, lo:hi]
                    )

            mu_b = bc_pool.tile([128, N], F32)
            nc.gpsimd.partition_broadcast(mu_b[:], mu_row[:])
            rstd_b = bc_pool.tile([128, N], F32)
            nc.gpsimd.partition_broadcast(rstd_b[:], rstd_row[:])

            for i in range(EC):
                nc.gpsimd.dma_start(wo[i][:], wo_d[i * 128 : (i + 1) * 128, :])

            # normalize: XN = (x - mu) * rstd  (LN affine folded into w_qkv).
            # Interleaved per chunk so XN[i] completes in consumption order
            # and the QKV pre-loop can start on chunk 0 immediately.
            with tc.tile_pool(name="lnt", bufs=2) as ln_pool:
                for i in range(EC):
                    tln = ln_pool.tile([128, N], F32, tag="lnt", name=f"lnt{i}")
                    nc.vector.tensor_tensor(
                        tln[:], xt[i][:], mu_b[:], ALU.subtract
                    )
                    nc.vector.tensor_tensor(XN[i][:], tln[:], rstd_b[:], ALU.mult)

        # ============ phase 2: merged QKV + attention + out-proj ============
        with (
            tc.tile_pool(name="proj_ps", bufs=1, space="PSUM") as proj_ps,
            tc.tile_pool(name="st_ps", bufs=2, space="PSUM") as st_ps,
            tc.tile_pool(name="ctx_ps", bufs=1, space="PSUM") as ctx_ps,
            tc.tile_pool(name="pt", bufs=24) as pt_pool,
            tc.tile_pool(name="stage", bufs=3) as stage_pool,
            tc.tile_pool(name="rr", bufs=2) as rr_pool,
            tc.tile_pool(name="rb", bufs=2) as rb_pool,
            tc.tile_pool(name="o_sb", bufs=2) as o_sb,
            tc.tile_pool(name="o_part", bufs=1) as o_part,
        ):
            OP = [
                o_part.tile([128, E], F16, tag=f"opart{t}", name=f"opart{t}")
                for t in range(TT)
            ]
            # ---- filler machinery: a stream of small independent PE jobs ----
            cur = {"ps": None}

            def qkt_chunk(ft, i):
                if i == 0:
                    cur["ps"] = proj_ps.tile(
                        [128, N], F32, tag="pps", name=f"qk{ft}"
                    )
                ps = cur["ps"]
                for hf in range(2):
                    sl = slice(hf * 512, hf * 512 + 512)
                    nc.tensor.matmul(
                        ps[:, sl],
                        wq[i][:, ft * 128 : ft * 128 + 128],
                        XN[i][:, sl],
                        start=(i == 0), stop=(i == EC - 1),
                    )
                if i == EC - 1:
                    bias = bqk[:, ft : ft + 1]
                    if ft < 6:
                        nc.vector.tensor_scalar(
                            QT[ft][:], ps[:], bias, 1.0 / np.sqrt(HD),
                            op0=ALU.add, op1=ALU.mult,
                        )
                    else:
                        nc.vector.tensor_scalar_add(KT[ft - 6][:], ps[:], bias)

            def v_chunk(tt, i):
                if i == 0:
                    cur["ps"] = proj_ps.tile(
                        [128, E], F32, tag="pps", name=f"vp{tt}"
                    )
                ps = cur["ps"]
                nc.tensor.matmul(
                    ps[:, 0:512],
                    XN[i][:, tt * 128 : tt * 128 + 128],
                    wq[i][:, 1536:2048],
                    start=(i == 0), stop=(i == EC - 1),
                )
                nc.tensor.matmul(
                    ps[:, 512:768],
                    XN[i][:, tt * 128 : tt * 128 + 128],
                    wq[i][:, 2048:2304],
                    start=(i == 0), stop=(i == EC - 1),
                )
                if i == EC - 1:
                    vt = V[tt]
                    v3 = vt[:].rearrange("p (h d) -> p h d", d=65)
                    nc.vector.tensor_tensor(
                        v3[:, :, 0:64],
                        ps[:].rearrange("p (h d) -> p h d", d=64),
                        bv[:].rearrange("p (h d) -> p h d", d=64),
                        ALU.add,
                    )
                    nc.vector.tensor_copy(
                        v3[:, :, 64:65],
                        nc.const_aps.tensor(1.0, (128, 12)).unsqueeze(-1),
                    )

            def out_chunk(tt, ecs):
                # one epoch: accumulate ec chunks `ecs` in psum, then fold
                # into the SBUF partial (or emit the final result)
                ps = proj_ps.tile([128, E], F32, tag="pps", name=f"op{tt}_{ecs[0]}")
                for j, i in enumerate(ecs):
                    nc.tensor.matmul(
                        ps[:, 0:512],
                        CT[i][:, tt * 128 : tt * 128 + 128],
                        wo[i][:, 0:512],
                        start=(j == 0), stop=(j == len(ecs) - 1),
                    )
                    nc.tensor.matmul(
                        ps[:, 512:768],
                        CT[i][:, tt * 128 : tt * 128 + 128],
                        wo[i][:, 512:768],
                        start=(j == 0), stop=(j == len(ecs) - 1),
                    )
                if ecs[0] == 0:
                    # first epoch: partial = psum + bias
                    nc.vector.tensor_tensor(OP[tt][:], ps[:], bo[:], ALU.add)
                elif ecs[-1] != EC - 1:
                    nc.vector.tensor_tensor(OP[tt][:], ps[:], OP[tt][:], ALU.add)
                else:
                    ot = o_sb.tile([128, E], F32, tag="osb", name=f"ot{tt}")
                    nc.vector.tensor_tensor(ot[:], ps[:], OP[tt][:], ALU.add)
                    nc.sync.dma_start(
                        out_d[tt * 128 : (tt + 1) * 128, :], ot[:]
                    )

            fillers = []

            def run_fillers(k):
                for _ in range(k):
                    if fillers:
                        fillers.pop(0)()

            # ---- per-head evacuate + normalize ----
            def evac_norm(h, cps):
                # stage the ctx+den PSUM to SBUF (frees the PSUM bank pair)
                stg = stage_pool.tile([65, N], F32, tag="stg", name=f"stg{h}")
                nc.vector.tensor_copy(stg[:], cps[:])
                # den row to partition 0 (cross-partition moves go via DMA)
                rr = rr_pool.tile([1, N], F32, tag="rr", name=f"rr{h}")
                nc.sync.dma_start(rr[:], stg[64:65, :])
                # 1/den (approx is plenty: ~18 correct bits)
                rri = rr_pool.tile([1, N], F32, tag="rri", name=f"rri{h}")
                nc.vector.reciprocal_approx_fast(rri[:], rr[:])
                rb = rb_pool.tile([64, N], F32, tag="rb", name=f"rb{h}")
                nc.gpsimd.partition_broadcast(rb[:], rri[:])
                pofs = (h % 2) * 64
                dest = CT[h // 2][pofs : pofs + 64, :]
                nc.vector.tensor_tensor(dest, stg[0:64, :], rb[:], ALU.mult)

            # ---- pre-loop: Q/K ftiles for pairs 0 and 1 ----
            for ft in (0, 6, 1, 7):
                for i in range(EC):
                    qkt_chunk(ft, i)

            # V projection + remaining Q/K ftiles paced as fillers
            for tt in range(TT):
                for i in range(EC):
                    fillers.append(lambda tt=tt, i=i: v_chunk(tt, i))
            for ft in (2, 8, 3, 9, 4, 10, 5, 11):
                for i in range(EC):
                    fillers.append(lambda ft=ft, i=i: qkt_chunk(ft, i))

            # ---- main attention loop over head pairs ----
            # At pair p: row-tiled concurrent score matmuls + exp for heads
            # (2p, 2p+1); PV for pair p-1 (h0 during kt 0-3, h1 during 4-7).
            prev_pts = None  # (pts_h0, pts_h1) of previous pair
            cps_cur = None
            for p in range(NP):
                h0, h1 = 2 * p, 2 * p + 1
                pts0, pts1 = [], []
                for kt in range(TT):
                    # --- PV work for the previous pair ---
                    if prev_pts is not None:
                        ph = 2 * (p - 1) + (0 if kt < 4 else 1)
                        ppts = prev_pts[0] if kt < 4 else prev_pts[1]
                        if kt == 0:
                            cps_cur = ctx_ps.tile(
                                [65, N], F32, tag="ctxps", name=f"cps{ph}"
                            )
                        elif kt == 4:
                            evac_norm(ph - 1, cps_cur)
                            cps_cur = ctx_ps.tile(
                                [65, N], F32, tag="ctxps", name=f"cps{ph}"
                            )
                        for kk in (2 * (kt % 4), 2 * (kt % 4) + 1):
                            vch = V[kk][:, 65 * ph : 65 * ph + 65]
                            for hf in range(2):
                                sl = slice(hf * 512, hf * 512 + 512)
                                nc.tensor.matmul(
                                    cps_cur[:, sl], vch, ppts[kk][:, sl],
                                    start=(kk == 0), stop=(kk == TT - 1),
                                )
                    # --- row-tiled score pair: h0 rows 0-63, h1 rows 64-127 ---
                    psA = st_ps.tile([128, N], F32, tag="stps", name=f"stA{p}_{kt}")
                    psB = st_ps.tile([128, N], F32, tag="stps", name=f"stB{p}_{kt}")
                    ksl = slice(kt * 128, kt * 128 + 128)
                    for hf in range(2):
                        sl = slice(hf * 512, hf * 512 + 512)
                        nc.tensor.matmul(
                            psA[:, sl],
                            KT[p][0:64, ksl], QT[p][0:64, sl],
                            start=True, stop=True, tile_position=(0, 0),
                        )
                        nc.tensor.matmul(
                            psB[:, sl],
                            KT[p][64:128, ksl], QT[p][64:128, sl],
                            start=True, stop=True, tile_position=(64, 0),
                        )
                    pt0 = pt_pool.tile([128, N], F16, tag="pt", name=f"pt{h0}_{kt}")
                    nc.scalar.activation(pt0[:], psA[:], ACTF.Exp, bias=negc[:])
                    pts0.append(pt0)
                    pt1 = pt_pool.tile([128, N], F16, tag="pt", name=f"pt{h1}_{kt}")
                    nc.scalar.activation(pt1[:], psB[:], ACTF.Exp, bias=negc[:])
                    pts1.append(pt1)
                    run_fillers(6 if p == 0 else 3)
                if prev_pts is not None:
                    evac_norm(2 * p - 1, cps_cur)
                    # out-proj epochs as CT chunks complete (CT[p-1] just done)
                    if p - 1 == 1:
                        for tt in range(TT):
                            fillers.append(lambda tt=tt: out_chunk(tt, (0, 1)))
                    elif p - 1 == 3:
                        for tt in range(TT):
                            fillers.append(lambda tt=tt: out_chunk(tt, (2, 3)))
                prev_pts = (pts0, pts1)

            # ---- drain: PV + evac/normalize for the last pair ----
            # CT[4] completed at the end of the main loop; its epoch keeps
            # the PE busy (and HAM warm) while the last heads normalize
            for tt in range(TT):
                fillers.append(lambda tt=tt: out_chunk(tt, (4,)))
            for ph in (H - 2, H - 1):
                ppts = prev_pts[0] if ph == H - 2 else prev_pts[1]
                # h11's ctx comes from the score-PSUM pool (idle in the
                # drain) so its PV needn't wait for h10's evacuation
                pool = ctx_ps if ph == H - 2 else st_ps
                cps_cur = pool.tile(
                    [65, N], F32,
                    tag="ctxps" if ph == H - 2 else "stps",
                    name=f"cps{ph}",
                )
                for kk in range(TT):
                    vch = V[kk][:, 65 * ph : 65 * ph + 65]
                    for hf in range(2):
                        sl = slice(hf * 512, hf * 512 + 512)
                        nc.tensor.matmul(
                            cps_cur[:, sl], vch, ppts[kk][:, sl],
                            start=(kk == 0), stop=(kk == TT - 1),
                        )
                    run_fillers(2)
                evac_norm(ph, cps_cur)
            run_fillers(len(fillers))
            # warmth keepers bridge the normalize chain of the last heads so
            # the final out-proj epoch runs at 2.4GHz
            wtail = st_ps.tile([128, 512], F32, tag="stps", name="wtail")
            for w in range(16):
                nc.tensor.matmul(
                    wtail[:], XN[w % EC][:, 0:128], wq[0][:, 0:512],
                    start=True, stop=True,
                )
            for tt in range(TT):
                out_chunk(tt, (5,))


def _prep_in_maps(x, ln_g, ln_b, w_qkv, b_qkv, w_out, b_out):
    x = np.asarray(x, np.float32)
    ln_g = np.asarray(ln_g, np.float32)
    ln_b = np.asarray(ln_b, np.float32)
    w_qkv = np.asarray(w_qkv, np.float32)
    b_qkv = np.asarray(b_qkv, np.float32)
    w_out = np.asarray(w_out, np.float32)
    b_out = np.asarray(b_out, np.float32)

    # Fold the LN affine into the packed projection:
    #   qkv = (xhat*g + b) @ W^T + b_qkv = xhat @ (W*g)^T + (W @ b + b_qkv)
    w_fold = w_qkv * ln_g[None, :]
    b_fold = b_qkv + w_qkv @ ln_b

    wqkvT = np.ascontiguousarray(w_fold.T.astype(np.float16))  # [E, 3E]
    woutT = np.ascontiguousarray(w_out.T.astype(np.float16))  # [E, E]
    bqk = np.ascontiguousarray(b_fold[:1536].reshape(12, 128).T)  # [128, 12]
    bv_b = np.ascontiguousarray(np.broadcast_to(b_fold[1536:], (128, E)))
    bo_b = np.ascontiguousarray(np.broadcast_to(b_out, (128, E)))

    in_maps = []
    for c in range(B):
        in_maps.append(
            {
                "xT": np.ascontiguousarray(x[c].T.astype(np.float16)),
                "wqkvT": wqkvT,
                "woutT": woutT,
                "bqk": bqk,
                "bv_b": bv_b,
                "bo_b": bo_b,
            }
        )
    return in_maps


def run(trace=False, **inputs):
    if "nc" not in _cache:
        _cache["nc"] = _build_kernel()
    nc = _cache["nc"]
    in_maps = _prep_in_maps(**inputs)
    res = bass_utils.run_bass_kernel_spmd(
        nc, in_maps, core_ids=list(range(B)), trace=trace
    )
    out = np.stack([res.results[c]["out"] for c in range(B)], axis=0)
    return out, res


def kernel(**inputs):
    out, _ = run(trace=False, **inputs)
    return out


if __name__ == "__main__":
    rng = np.random.default_rng(0)
    inputs = {
        "x": rng.standard_normal((B, N, E), dtype=np.float32),
        "ln_g": np.ones(E, np.float32),
        "ln_b": np.zeros(E, np.float32),
        "w_qkv": rng.standard_normal((F3, E), dtype=np.float32) / np.sqrt(E),
        "b_qkv": np.zeros(F3, np.float32),
        "w_out": rng.standard_normal((E, E), dtype=np.float32) / np.sqrt(E),
        "b_out": np.zeros(E, np.float32),
    }
    y = kernel(**inputs)
    print("out shape", y.shape, "mean", float(np.abs(y).mean()))


# revision 33
# speedup vs baseline: 1.0827x; 1.0827x over previous
"""Trainium2 Bass kernel for a pre-LN multi-head self-attention block.

Problem: y = out_proj(MHA(LayerNorm(x))) with B=8, N=1024, E=768, H=12.

Sharding: pure data-parallel — batch element b runs on core b (8 cores, no
collectives). Host-side prep: transposes, fp16 weight conversion, and folding
the LN affine into the QKV projection (w_qkv·diag(g), b_qkv + W@ln_b), so the
on-device LN is just (x - mu)·rstd.

Per-core design (feature-major; no PE transposes anywhere):
  1. LayerNorm stats via ones-vector matmuls (sum / sum-of-squares),
     rstd = exp(-0.5*ln(var+eps)) on ACT, normalize = 2 DVE ops per chunk.
  2. QKV projection in fp16. Q^T/K^T feature-major [f, tok] with a head PAIR
     packed per 128-partition tile (h0 in partitions 0-63, h1 in 64-127);
     V token-major with a ones column per 65-wide head slab (PV's extra
     output row accumulates softmax denominators for free). Q pre-scaled.
  3. Attention processed in HEAD PAIRS: the two heads' score matmuls
     (contraction = head_dim = 64) are emitted back-to-back as 64x128-mode
     row tiles (tile_position (0,0) and (64,0)) so they run CONCURRENTLY in
     the PE array — 2x score throughput vs sequential heads. exp(S^T - 9) on
     ACT per head; ACT is the near-critical engine so everything else hides
     under it.
  4. PV for the previous pair is interleaved into the current pair's steps
     (h0 during kt 0-3, h1 during kt 4-7; ctx PSUM recycled in between).
     Per-head normalize right after evacuation: reciprocal_approx_fast on the
     denominator row, gpsimd broadcast, one DVE multiply.
  5. V projection and the remaining QKV ftiles run as PE "fillers" during the
     ACT-bound attention steps; out-projection epochs are appended as their
     CT chunks complete. Keeps the PE dense (HAM stays at 2.4 GHz).
"""

import sys

sys.path.insert(0, "/opt/trn_rl_repo")

import numpy as np

import concourse.bass as bass
import concourse.tile as tile
from concourse import bacc, mybir
from concourse import bass_utils

F32 = mybir.dt.float32
F32R = mybir.dt.float32r
F16 = mybir.dt.float16
ALU = mybir.AluOpType
ACTF = mybir.ActivationFunctionType

B, N, E, H, HD = 8, 1024, 768, 12, 64
F3 = 3 * E  # 2304
EC = E // 128  # 6 feature chunks
TT = N // 128  # 8 token tiles
NP = H // 2  # 6 head pairs
EPS = 1e-5

_cache = {}


def _build_kernel():
    nc = bacc.Bacc(
        "TRN2", target_bir_lowering=False, debug=False, num_devices=B
    )

    xT_d = nc.dram_tensor("xT", [E, N], F16, kind="ExternalInput").ap()
    wq_d = nc.dram_tensor("wqkvT", [E, F3], F16, kind="ExternalInput").ap()
    wo_d = nc.dram_tensor("woutT", [E, E], F16, kind="ExternalInput").ap()
    bqk_d = nc.dram_tensor("bqk", [128, 12], F32, kind="ExternalInput").ap()
    bv_d = nc.dram_tensor("bv_b", [128, E], F32, kind="ExternalInput").ap()
    bo_d = nc.dram_tensor("bo_b", [128, E], F32, kind="ExternalInput").ap()
    out_d = nc.dram_tensor("out", [N, E], F32, kind="ExternalOutput").ap()

    with tile.TileContext(nc) as tc:
        _emit(nc, tc, xT_d, wq_d, wo_d, bqk_d, bv_d, bo_d, out_d)

    nc.compile()
    return nc


def _emit(nc, tc, xT_d, wq_d, wo_d, bqk_d, bv_d, bo_d, out_d):
    from contextlib import ExitStack

    with ExitStack() as octx:
        # ---- long-lived pools ----
        cpool = octx.enter_context(tc.tile_pool(name="consts", bufs=1))
        qt_pool = octx.enter_context(tc.tile_pool(name="qt", bufs=1))
        kt_pool = octx.enter_context(tc.tile_pool(name="kt", bufs=1))
        v_pool = octx.enter_context(tc.tile_pool(name="v", bufs=1))
        xn_pool = octx.enter_context(tc.tile_pool(name="xn", bufs=1))
        wq_pool = octx.enter_context(tc.tile_pool(name="wq", bufs=1))
        wo_pool = octx.enter_context(tc.tile_pool(name="wo", bufs=1))
        ct_pool = octx.enter_context(tc.tile_pool(name="ctxT", bufs=1))

        # consts + w_out ride the gpsimd SWDGE queue; x keeps the sync queue
        # and w_qkv the scalar HWDGE queue so the three input streams overlap.
        bqk = cpool.tile([128, 12], F32)
        nc.gpsimd.dma_start(bqk[:], bqk_d[:])
        bv = cpool.tile([128, E], F32)
        nc.gpsimd.dma_start(bv[:], bv_d[:])
        bo = cpool.tile([128, E], F32)
        nc.gpsimd.dma_start(bo[:], bo_d[:])
        ones_col = cpool.tile([128, 1], F16)
        nc.vector.tensor_copy(ones_col[:], nc.const_aps.tensor(1.0, (128, 1)))
        # constant shift inside exp: keeps unnormalized P within fp16 range
        # (softmax is invariant to it; denominators scale uniformly)
        negc = cpool.tile([128, 1], F32)
        nc.vector.memset(negc[:], -9.0)

        QT = [qt_pool.tile([128, N], F16, tag=f"qt{i}", name=f"qt{i}") for i in range(EC)]
        KT = [kt_pool.tile([128, N], F16, tag=f"kt{i}", name=f"kt{i}") for i in range(EC)]
        VW = 65 * H  # 780: 64 features + ones column per head
        V = [v_pool.tile([128, VW], F16, tag=f"v{i}", name=f"v{i}") for i in range(TT)]
        XN = [xn_pool.tile([128, N], F16, tag=f"xn{i}", name=f"xn{i}") for i in range(EC)]
        CT = [ct_pool.tile([128, N], F16, tag=f"ct{i}", name=f"ct{i}") for i in range(EC)]
        wq = [wq_pool.tile([128, F3], F16, tag=f"w{i}", name=f"w{i}") for i in range(EC)]
        wo = [wo_pool.tile([128, E], F16, tag=f"wo{i}", name=f"wo{i}") for i in range(EC)]

        # ================= phase 1: load x, LN =================
        with (
            tc.tile_pool(name="xt", bufs=1) as xt_pool,
            tc.tile_pool(name="tmp", bufs=2) as tmp_pool,
            tc.tile_pool(name="rows", bufs=3) as row_pool,
            tc.tile_pool(name="bcast", bufs=1) as bc_pool,
        ):
            # ALL input tensors ride the sync queue: same-queue transfers
            # execute in issue order, which is the only reliable way to
            # prioritize (cross-queue transfers share the DMA fabric
            # concurrently, and the Tile scheduler hoists DMA issues).
            # Priority: x (gates LN) > w_qkv pre-loop ftile columns
            # (Q0,Q1 = 0:256, K0,K1 = 768:1024) > V columns > rest > w_out.
            xt = [xt_pool.tile([128, N], F16, tag=f"x{i}", name=f"x{i}") for i in range(EC)]
            for i in range(EC):
                nc.sync.dma_start(xt[i][:], xT_d[i * 128 : (i + 1) * 128, :])
            for lo, hi in ((0, 256), (768, 1024), (1536, 2304), (256, 768), (1024, 1536)):
                for i in range(EC):
                    nc.sync.dma_start(
                        wq[i][:, lo:hi], wq_d[i * 128 : (i + 1) * 128, lo:hi]
                    )
            for i in range(EC):
                nc.sync.dma_start(wo[i][:], wo_d[i * 128 : (i + 1) * 128, :])

            with tc.tile_pool(name="stats_ps", bufs=1, space="PSUM") as stats_ps:
                ps_sum = stats_ps.tile([1, N], F32)
                ps_sq = stats_ps.tile([1, N], F32)
                for i in range(EC):
                    xsq = tmp_pool.tile([128, N], F16, tag="tmp", name="xsq")
                    nc.vector.tensor_tensor(
                        xsq[:], xt[i][:], xt[i][:], ALU.mult,
                    )
                    st, sp = i == 0, i == EC - 1
                    for hf in range(2):
                        sl = slice(hf * 512, hf * 512 + 512)
                        nc.tensor.matmul(
                            ps_sum[:, sl], ones_col[:], xt[i][:, sl],
                            start=st, stop=sp,
                        )
                        nc.tensor.matmul(
                            ps_sq[:, sl], ones_col[:], xsq[:, sl],
                            start=st, stop=sp,
                        )
                    # density keepers on the just-arrived chunk: the stats
                    # stream alone is too sparse for HAM to unthrottle
                    wscr = stats_ps.tile([1, 512], F32, tag="wscr")
                    for w in range(2):
                        nc.tensor.matmul(
                            wscr[:], ones_col[:], xt[i][:, 0:512],
                            start=True, stop=True,
                        )
                # warmth keepers: harmless matmuls that bridge the PE-idle
                # window while mu/rstd/normalize run, so HAM stays at 2.4GHz
                # when the QKV projection starts
                wscr = stats_ps.tile([1, 512], F32, tag="wscr")
                for w in range(24):
                    nc.tensor.matmul(
                        wscr[:], ones_col[:], xt[w % EC][:, 0:512],
                        start=True, stop=True,
                    )

                mu_row = row_pool.tile([1, N], F32, tag="row", name="mu_row")
                nc.vector.tensor_scalar_mul(mu_row[:], ps_sum[:], 1.0 / E)
                msq_row = row_pool.tile([1, N], F32, tag="row", name="msq_row")
                nc.vector.tensor_tensor(msq_row[:], mu_row[:], mu_row[:], ALU.mult)
                var_row = row_pool.tile([1, N], F32, tag="row", name="var_row")
                nc.vector.scalar_tensor_tensor(
                    var_row[:], ps_sq[:], 1.0 / E, msq_row[:],
                    ALU.mult, ALU.subtract,
                )
            eps_ap = row_pool.tile([1, 1], F32)
            nc.vector.memset(eps_ap[:], EPS)
            # rstd = exp(-0.5 * ln(var + eps)) — both on ACT (fast row ops)
            lnv_row = row_pool.tile([1, N], F32, tag="row", name="lnv_row")
            nc.scalar.activation(lnv_row[:], var_row[:], ACTF.Ln, bias=eps_ap[:])
            rstd_row = row_pool.tile([1, N], F32, tag="row", name="rstd_row")
            nc.scalar.activation(rstd_row[:], lnv_row[:], ACTF.Exp, scale=-0.5)

            mu_b = bc_pool.tile([128, N], F32)
            nc.gpsimd.partition_broadcast(mu_b[:], mu_row[:])
            rstd_b = bc_pool.tile([128, N], F32)
            nc.gpsimd.partition_broadcast(rstd_b[:], rstd_row[:])

            # normalize: XN = (x - mu) * rstd  (LN affine folded into w_qkv).
            # Interleaved per chunk so XN[i] completes in consumption order
            # and the QKV pre-loop can start on chunk 0 immediately.
            with tc.tile_pool(name="lnt", bufs=2) as ln_pool:
                for i in range(EC):
                    tln = ln_pool.tile([128, N], F32, tag="lnt", name=f"lnt{i}")
                    nc.vector.tensor_tensor(
                        tln[:], xt[i][:], mu_b[:], ALU.subtract
                    )
                    nc.vector.tensor_tensor(XN[i][:], tln[:], rstd_b[:], ALU.mult)

        # ============ phase 2: merged QKV + attention + out-proj ============
        with (
            tc.tile_pool(name="proj_ps", bufs=1, space="PSUM") as proj_ps,
            tc.tile_pool(name="st_ps", bufs=2, space="PSUM") as st_ps,
            tc.tile_pool(name="ctx_ps", bufs=1, space="PSUM") as ctx_ps,
            tc.tile_pool(name="pt", bufs=24) as pt_pool,
            tc.tile_pool(name="stage", bufs=3) as stage_pool,
            tc.tile_pool(name="rr", bufs=2) as rr_pool,
            tc.tile_pool(name="rb", bufs=2) as rb_pool,
            tc.tile_pool(name="o_sb", bufs=2) as o_sb,
            tc.tile_pool(name="o_part", bufs=1) as o_part,
        ):
            OP = [
                o_part.tile([128, E], F16, tag=f"opart{t}", name=f"opart{t}")
                for t in range(TT)
            ]
            # ---- filler machinery: a stream of small independent PE jobs ----
            cur = {"ps": None}

            def qkt_chunk(ft, i, ps=None):
                if ps is None:
                    if i == 0:
                        cur["ps"] = proj_ps.tile(
                            [128, N], F32, tag="pps", name=f"qk{ft}"
                        )
                    ps = cur["ps"]
                for hf in range(2):
                    sl = slice(hf * 512, hf * 512 + 512)
                    nc.tensor.matmul(
                        ps[:, sl],
                        wq[i][:, ft * 128 : ft * 128 + 128],
                        XN[i][:, sl],
                        start=(i == 0), stop=(i == EC - 1),
                    )
                if i == EC - 1:
                    bias = bqk[:, ft : ft + 1]
                    if ft < 6:
                        nc.vector.tensor_scalar(
                            QT[ft][:], ps[:], bias, 1.0 / np.sqrt(HD),
                            op0=ALU.add, op1=ALU.mult,
                        )
                    else:
                        nc.vector.tensor_scalar_add(KT[ft - 6][:], ps[:], bias)

            def v_chunk(tt, i):
                if i == 0:
                    cur["ps"] = proj_ps.tile(
                        [128, E], F32, tag="pps", name=f"vp{tt}"
                    )
                ps = cur["ps"]
                nc.tensor.matmul(
                    ps[:, 0:512],
                    XN[i][:, tt * 128 : tt * 128 + 128],
                    wq[i][:, 1536:2048],
                    start=(i == 0), stop=(i == EC - 1),
                )
                nc.tensor.matmul(
                    ps[:, 512:768],
                    XN[i][:, tt * 128 : tt * 128 + 128],
                    wq[i][:, 2048:2304],
                    start=(i == 0), stop=(i == EC - 1),
                )
                if i == EC - 1:
                    vt = V[tt]
                    v3 = vt[:].rearrange("p (h d) -> p h d", d=65)
                    nc.vector.tensor_tensor(
                        v3[:, :, 0:64],
                        ps[:].rearrange("p (h d) -> p h d", d=64),
                        bv[:].rearrange("p (h d) -> p h d", d=64),
                        ALU.add,
                    )
                    nc.vector.tensor_copy(
                        v3[:, :, 64:65],
                        nc.const_aps.tensor(1.0, (128, 12)).unsqueeze(-1),
                    )

            def out_chunk(tt, ecs):
                # one epoch: accumulate ec chunks `ecs` in psum, then fold
                # into the SBUF partial (or emit the final result)
                ps = proj_ps.tile([128, E], F32, tag="pps", name=f"op{tt}_{ecs[0]}")
                for j, i in enumerate(ecs):
                    nc.tensor.matmul(
                        ps[:, 0:512],
                        CT[i][:, tt * 128 : tt * 128 + 128],
                        wo[i][:, 0:512],
                        start=(j == 0), stop=(j == len(ecs) - 1),
                    )
                    nc.tensor.matmul(
                        ps[:, 512:768],
                        CT[i][:, tt * 128 : tt * 128 + 128],
                        wo[i][:, 512:768],
                        start=(j == 0), stop=(j == len(ecs) - 1),
                    )
                if ecs[0] == 0:
                    # first epoch: partial = psum + bias
                    nc.vector.tensor_tensor(OP[tt][:], ps[:], bo[:], ALU.add)
                elif ecs[-1] != EC - 1:
                    nc.vector.tensor_tensor(OP[tt][:], ps[:], OP[tt][:], ALU.add)
                else:
                    ot = o_sb.tile([128, E], F32, tag="osb", name=f"ot{tt}")
                    nc.vector.tensor_tensor(ot[:], ps[:], OP[tt][:], ALU.add)
                    nc.sync.dma_start(
                        out_d[tt * 128 : (tt + 1) * 128, :], ot[:]
                    )

            fillers = []

            def run_fillers(k):
                for _ in range(k):
                    if fillers:
                        fillers.pop(0)()

            # ---- per-head evacuate + normalize ----
            def evac_norm(h, cps):
                # stage the ctx+den PSUM to SBUF (frees the PSUM bank pair)
                stg = stage_pool.tile([65, N], F32, tag="stg", name=f"stg{h}")
                nc.vector.tensor_copy(stg[:], cps[:])
                # den row to partition 0 (cross-partition moves go via DMA)
                rr = rr_pool.tile([1, N], F32, tag="rr", name=f"rr{h}")
                nc.sync.dma_start(rr[:], stg[64:65, :])
                # 1/den (approx is plenty: ~18 correct bits)
                rri = rr_pool.tile([1, N], F32, tag="rri", name=f"rri{h}")
                nc.vector.reciprocal_approx_fast(rri[:], rr[:])
                rb = rb_pool.tile([64, N], F32, tag="rb", name=f"rb{h}")
                nc.gpsimd.partition_broadcast(rb[:], rri[:])
                pofs = (h % 2) * 64
                dest = CT[h // 2][pofs : pofs + 64, :]
                nc.vector.tensor_tensor(dest, stg[0:64, :], rb[:], ALU.mult)

            # ---- pre-loop: Q/K ftiles for pairs 0 and 1 ----
            # i-major so each chunk's matmuls run as its XN lands (the LN
            # chain produces XN[i] every ~2us; ft-major would stall the PE
            # long enough for HAM to re-throttle). Four concurrent PSUM
            # accumulators borrowed from the still-idle attention pools.
            pre_ps = {
                0: proj_ps.tile([128, N], F32, tag="pps", name="pre0"),
                6: st_ps.tile([128, N], F32, tag="stps", name="pre6"),
                1: st_ps.tile([128, N], F32, tag="stps", name="pre1"),
                7: ctx_ps.tile([128, N], F32, tag="ctxps", name="pre7"),
            }
            for i in range(EC):
                for ft in (0, 6, 1, 7):
                    qkt_chunk(ft, i, ps=pre_ps[ft])

            # V projection + remaining Q/K ftiles paced as fillers
            for tt in range(TT):
                for i in range(EC):
                    fillers.append(lambda tt=tt, i=i: v_chunk(tt, i))
            for ft in (2, 8, 3, 9, 4, 10, 5, 11):
                for i in range(EC):
                    fillers.append(lambda ft=ft, i=i: qkt_chunk(ft, i))

            # ---- main attention loop over head pairs ----
            # At pair p: row-tiled concurrent score matmuls + exp for heads
            # (2p, 2p+1); PV for pair p-1 (h0 during kt 0-3, h1 during 4-7).
            prev_pts = None  # (pts_h0, pts_h1) of previous pair
            cps_cur = None
            for p in range(NP):
                h0, h1 = 2 * p, 2 * p + 1
                pts0, pts1 = [], []
                for kt in range(TT):
                    # --- PV work for the previous pair ---
                    if prev_pts is not None:
                        ph = 2 * (p - 1) + (0 if kt < 4 else 1)
                        ppts = prev_pts[0] if kt < 4 else prev_pts[1]
                        if kt == 0:
                            cps_cur = ctx_ps.tile(
                                [65, N], F32, tag="ctxps", name=f"cps{ph}"
                            )
                        elif kt == 4:
                            evac_norm(ph - 1, cps_cur)
                            cps_cur = ctx_ps.tile(
                                [65, N], F32, tag="ctxps", name=f"cps{ph}"
                            )
                        for kk in (2 * (kt % 4), 2 * (kt % 4) + 1):
                            vch = V[kk][:, 65 * ph : 65 * ph + 65]
                            for hf in range(2):
                                sl = slice(hf * 512, hf * 512 + 512)
                                nc.tensor.matmul(
                                    cps_cur[:, sl], vch, ppts[kk][:, sl],
                                    start=(kk == 0), stop=(kk == TT - 1),
                                )
                    # --- row-tiled score pair: h0 rows 0-63, h1 rows 64-127 ---
                    psA = st_ps.tile([128, N], F32, tag="stps", name=f"stA{p}_{kt}")
                    psB = st_ps.tile([128, N], F32, tag="stps", name=f"stB{p}_{kt}")
                    ksl = slice(kt * 128, kt * 128 + 128)
                    for hf in range(2):
                        sl = slice(hf * 512, hf * 512 + 512)
                        nc.tensor.matmul(
                            psA[:, sl],
                            KT[p][0:64, ksl], QT[p][0:64, sl],
                            start=True, stop=True, tile_position=(0, 0),
                        )
                        nc.tensor.matmul(
                            psB[:, sl],
                            KT[p][64:128, ksl], QT[p][64:128, sl],
                            start=True, stop=True, tile_position=(64, 0),
                        )
                    pt0 = pt_pool.tile([128, N], F16, tag="pt", name=f"pt{h0}_{kt}")
                    nc.scalar.activation(pt0[:], psA[:], ACTF.Exp, bias=negc[:])
                    pts0.append(pt0)
                    pt1 = pt_pool.tile([128, N], F16, tag="pt", name=f"pt{h1}_{kt}")
                    nc.scalar.activation(pt1[:], psB[:], ACTF.Exp, bias=negc[:])
                    pts1.append(pt1)
                    run_fillers(6 if p == 0 else 3)
                if prev_pts is not None:
                    evac_norm(2 * p - 1, cps_cur)
                    # out-proj epochs as CT chunks complete (CT[p-1] just done)
                    if p - 1 == 1:
                        for tt in range(TT):
                            fillers.append(lambda tt=tt: out_chunk(tt, (0, 1)))
                    elif p - 1 == 3:
                        for tt in range(TT):
                            fillers.append(lambda tt=tt: out_chunk(tt, (2, 3)))
                prev_pts = (pts0, pts1)

            # ---- drain: PV + evac/normalize for the last pair ----
            # CT[4] completed at the end of the main loop; its epoch keeps
            # the PE busy (and HAM warm) while the last heads normalize
            for tt in range(TT):
                fillers.append(lambda tt=tt: out_chunk(tt, (4,)))
            for ph in (H - 2, H - 1):
                ppts = prev_pts[0] if ph == H - 2 else prev_pts[1]
                # h11's ctx comes from the score-PSUM pool (idle in the
                # drain) so its PV needn't wait for h10's evacuation
                pool = ctx_ps if ph == H - 2 else st_ps
                cps_cur = pool.tile(
                    [65, N], F32,
                    tag="ctxps" if ph == H - 2 else "stps",
                    name=f"cps{ph}",
                )
                for kk in range(TT):
                    vch = V[kk][:, 65 * ph : 65 * ph + 65]
                    for hf in range(2):
                        sl = slice(hf * 512, hf * 512 + 512)
                        nc.tensor.matmul(
                            cps_cur[:, sl], vch, ppts[kk][:, sl],
                            start=(kk == 0), stop=(kk == TT - 1),
                        )
                    run_fillers(2)
                evac_norm(ph, cps_cur)
            run_fillers(len(fillers))
            # warmth keepers bridge the normalize chain of the last heads so
            # the final out-proj epoch runs at 2.4GHz
            wtail = st_ps.tile([128, 512], F32, tag="stps", name="wtail")
            for w in range(16):
                nc.tensor.matmul(
                    wtail[:], XN[w % EC][:, 0:128], wq[0][:, 0:512],
                    start=True, stop=True,
                )
            for tt in range(TT):
                out_chunk(tt, (5,))


def _prep_in_maps(x, ln_g, ln_b, w_qkv, b_qkv, w_out, b_out):
    x = np.asarray(x, np.float32)
    ln_g = np.asarray(ln_g, np.float32)
    ln_b = np.asarray(ln_b, np.float32)
    w_qkv = np.asarray(w_qkv, np.float32)
    b_qkv = np.asarray(b_qkv, np.float32)
    w_out = np.asarray(w_out, np.float32)
    b_out = np.asarray(b_out, np.float32)

    # Fold the LN affine into the packed projection:
    #   qkv = (xhat*g + b) @ W^T + b_qkv = xhat @ (W*g)^T + (W @ b + b_qkv)
    w_fold = w_qkv * ln_g[None, :]
    b_fold = b_qkv + w_qkv @ ln_b

    wqkvT = np.ascontiguousarray(w_fold.T.astype(np.float16))  # [E, 3E]
    woutT = np.ascontiguousarray(w_out.T.astype(np.float16))  # [E, E]
    bqk = np.ascontiguousarray(b_fold[:1536].reshape(12, 128).T)  # [128, 12]
    bv_b = np.ascontiguousarray(np.broadcast_to(b_fold[1536:], (128, E)))
    bo_b = np.ascontiguousarray(np.broadcast_to(b_out, (128, E)))

    in_maps = []
    for c in range(B):
        in_maps.append(
            {
                "xT": np.ascontiguousarray(x[c].T.astype(np.float16)),
                "wqkvT": wqkvT,
                "woutT": woutT,
                "bqk": bqk,
                "bv_b": bv_b,
                "bo_b": bo_b,
            }
        )
    return in_maps


def run(trace=False, **inputs):
    if "nc" not in _cache:
        _cache["nc"] = _build_kernel()
    nc = _cache["nc"]
    in_maps = _prep_in_maps(**inputs)
    res = bass_utils.run_bass_kernel_spmd(
        nc, in_maps, core_ids=list(range(B)), trace=trace
    )
    out = np.stack([res.results[c]["out"] for c in range(B)], axis=0)
    return out, res


def kernel(**inputs):
    out, _ = run(trace=False, **inputs)
    return out


if __name__ == "__main__":
    rng = np.random.default_rng(0)
    inputs = {
        "x": rng.standard_normal((B, N, E), dtype=np.float32),
        "ln_g": np.ones(E, np.float32),
        "ln_b": np.zeros(E, np.float32),
        "w_qkv": rng.standard_normal((F3, E), dtype=np.float32) / np.sqrt(E),
        "b_qkv": np.zeros(F3, np.float32),
        "w_out": rng.standard_normal((E, E), dtype=np.float32) / np.sqrt(E),
        "b_out": np.zeros(E, np.float32),
    }
    y = kernel(**inputs)
    print("out shape", y.shape, "mean", float(np.abs(y).mean()))


# revision 37
# speedup vs baseline: 1.0981x; 1.0143x over previous
"""Trainium2 Bass kernel for a pre-LN multi-head self-attention block.

Problem: y = out_proj(MHA(LayerNorm(x))) with B=8, N=1024, E=768, H=12.

Sharding: pure data-parallel — batch element b runs on core b (8 cores, no
collectives). Host-side prep: transposes, fp16 weight conversion, and folding
the LN affine into the QKV projection (w_qkv·diag(g), b_qkv + W@ln_b), so the
on-device LN is just (x - mu)·rstd.

Per-core design (feature-major; no PE transposes anywhere):
  1. LayerNorm stats via ones-vector matmuls (sum / sum-of-squares),
     rstd = exp(-0.5*ln(var+eps)) on ACT, normalize = 2 DVE ops per chunk.
  2. QKV projection in fp16. Q^T/K^T feature-major [f, tok] with a head PAIR
     packed per 128-partition tile (h0 in partitions 0-63, h1 in 64-127);
     V token-major with a ones column per 65-wide head slab (PV's extra
     output row accumulates softmax denominators for free). Q pre-scaled.
  3. Attention processed in HEAD PAIRS: the two heads' score matmuls
     (contraction = head_dim = 64) are emitted back-to-back as 64x128-mode
     row tiles (tile_position (0,0) and (64,0)) so they run CONCURRENTLY in
     the PE array — 2x score throughput vs sequential heads. exp(S^T - 9) on
     ACT per head; ACT is the near-critical engine so everything else hides
     under it.
  4. PV for the previous pair is interleaved into the current pair's steps
     (h0 during kt 0-3, h1 during kt 4-7; ctx PSUM recycled in between).
     Per-head normalize right after evacuation: reciprocal_approx_fast on the
     denominator row, gpsimd broadcast, one DVE multiply.
  5. V projection and the remaining QKV ftiles run as PE "fillers" during the
     ACT-bound attention steps; out-projection epochs are appended as their
     CT chunks complete. Keeps the PE dense (HAM stays at 2.4 GHz).
"""

import sys

sys.path.insert(0, "/opt/trn_rl_repo")

import numpy as np

import concourse.bass as bass
import concourse.tile as tile
from concourse import bacc, mybir
from concourse import bass_utils

F32 = mybir.dt.float32
F32R = mybir.dt.float32r
F16 = mybir.dt.float16
ALU = mybir.AluOpType
ACTF = mybir.ActivationFunctionType

B, N, E, H, HD = 8, 1024, 768, 12, 64
F3 = 3 * E  # 2304
EC = E // 128  # 6 feature chunks
TT = N // 128  # 8 token tiles
NP = H // 2  # 6 head pairs
EPS = 1e-5

_cache = {}


def _build_kernel():
    nc = bacc.Bacc(
        "TRN2", target_bir_lowering=False, debug=False, num_devices=B
    )

    xT_d = nc.dram_tensor("xT", [E, N], F16, kind="ExternalInput").ap()
    wq_d = nc.dram_tensor("wqkvT", [E, F3], F16, kind="ExternalInput").ap()
    wo_d = nc.dram_tensor("woutT", [E, E], F16, kind="ExternalInput").ap()
    bqk_d = nc.dram_tensor("bqk", [128, 12], F32, kind="ExternalInput").ap()
    bv_d = nc.dram_tensor("bv_b", [128, E], F32, kind="ExternalInput").ap()
    bo_d = nc.dram_tensor("bo_b", [128, E], F32, kind="ExternalInput").ap()
    out_d = nc.dram_tensor("out", [N, E], F32, kind="ExternalOutput").ap()

    with tile.TileContext(nc) as tc:
        _emit(nc, tc, xT_d, wq_d, wo_d, bqk_d, bv_d, bo_d, out_d)

    nc.compile()
    return nc


def _emit(nc, tc, xT_d, wq_d, wo_d, bqk_d, bv_d, bo_d, out_d):
    from contextlib import ExitStack

    with ExitStack() as octx:
        # ---- long-lived pools ----
        cpool = octx.enter_context(tc.tile_pool(name="consts", bufs=1))
        qt_pool = octx.enter_context(tc.tile_pool(name="qt", bufs=1))
        kt_pool = octx.enter_context(tc.tile_pool(name="kt", bufs=1))
        v_pool = octx.enter_context(tc.tile_pool(name="v", bufs=1))
        xn_pool = octx.enter_context(tc.tile_pool(name="xn", bufs=1))
        wq_pool = octx.enter_context(tc.tile_pool(name="wq", bufs=1))
        wo_pool = octx.enter_context(tc.tile_pool(name="wo", bufs=1))
        ct_pool = octx.enter_context(tc.tile_pool(name="ctxT", bufs=1))

        # consts + w_out ride the gpsimd SWDGE queue; x keeps the sync queue
        # and w_qkv the scalar HWDGE queue so the three input streams overlap.
        bqk = cpool.tile([128, 12], F32)
        nc.gpsimd.dma_start(bqk[:], bqk_d[:])
        bv = cpool.tile([128, E], F32)
        nc.gpsimd.dma_start(bv[:], bv_d[:])
        bo = cpool.tile([128, E], F32)
        nc.gpsimd.dma_start(bo[:], bo_d[:])
        ones_col = cpool.tile([128, 1], F16)
        nc.vector.tensor_copy(ones_col[:], nc.const_aps.tensor(1.0, (128, 1)))
        # constant shift inside exp: keeps unnormalized P within fp16 range
        # (softmax is invariant to it; denominators scale uniformly)
        negc = cpool.tile([128, 1], F32)
        nc.vector.memset(negc[:], -9.0)

        QT = [qt_pool.tile([128, N], F16, tag=f"qt{i}", name=f"qt{i}") for i in range(EC)]
        KT = [kt_pool.tile([128, N], F16, tag=f"kt{i}", name=f"kt{i}") for i in range(EC)]
        VW = 65 * H  # 780: 64 features + ones column per head
        V = [v_pool.tile([128, VW], F16, tag=f"v{i}", name=f"v{i}") for i in range(TT)]
        XN = [xn_pool.tile([128, N], F16, tag=f"xn{i}", name=f"xn{i}") for i in range(EC)]
        CT = [ct_pool.tile([128, N], F16, tag=f"ct{i}", name=f"ct{i}") for i in range(EC)]
        wq = [wq_pool.tile([128, F3], F16, tag=f"w{i}", name=f"w{i}") for i in range(EC)]
        wo = [wo_pool.tile([128, E], F16, tag=f"wo{i}", name=f"wo{i}") for i in range(EC)]

        # ================= phase 1: load x, LN =================
        with (
            tc.tile_pool(name="xt", bufs=1) as xt_pool,
            tc.tile_pool(name="tmp", bufs=2) as tmp_pool,
            tc.tile_pool(name="rows", bufs=3) as row_pool,
            tc.tile_pool(name="bcast", bufs=1) as bc_pool,
        ):
            # ALL input tensors ride the sync queue: same-queue transfers
            # execute in issue order, which is the only reliable way to
            # prioritize (cross-queue transfers share the DMA fabric
            # concurrently, and the Tile scheduler hoists DMA issues).
            # Priority: x (gates LN) > w_qkv pre-loop ftile columns
            # (Q0,Q1 = 0:256, K0,K1 = 768:1024) > V columns > rest > w_out.
            xt = [xt_pool.tile([128, N], F16, tag=f"x{i}", name=f"x{i}") for i in range(EC)]
            for i in range(EC):
                nc.sync.dma_start(xt[i][:], xT_d[i * 128 : (i + 1) * 128, :])
            for lo, hi in ((0, 256), (768, 1024), (1536, 2304), (256, 768), (1024, 1536)):
                for i in range(EC):
                    nc.sync.dma_start(
                        wq[i][:, lo:hi], wq_d[i * 128 : (i + 1) * 128, lo:hi]
                    )
            for i in range(EC):
                nc.sync.dma_start(wo[i][:], wo_d[i * 128 : (i + 1) * 128, :])

            with tc.tile_pool(name="stats_ps", bufs=1, space="PSUM") as stats_ps:
                ps_sum = stats_ps.tile([1, N], F32)
                ps_sq = stats_ps.tile([1, N], F32)
                for i in range(EC):
                    xsq = tmp_pool.tile([128, N], F16, tag="tmp", name="xsq")
                    nc.vector.tensor_tensor(
                        xsq[:], xt[i][:], xt[i][:], ALU.mult,
                    )
                    st, sp = i == 0, i == EC - 1
                    for hf in range(2):
                        sl = slice(hf * 512, hf * 512 + 512)
                        nc.tensor.matmul(
                            ps_sum[:, sl], ones_col[:], xt[i][:, sl],
                            start=st, stop=sp,
                        )
                        nc.tensor.matmul(
                            ps_sq[:, sl], ones_col[:], xsq[:, sl],
                            start=st, stop=sp,
                        )
                    # density keepers on the just-arrived chunk: the stats
                    # stream alone is too sparse for HAM to unthrottle
                    wscr = stats_ps.tile([1, 512], F32, tag="wscr")
                    for w in range(2):
                        nc.tensor.matmul(
                            wscr[:], ones_col[:], xt[i][:, 0:512],
                            start=True, stop=True,
                        )
                # warmth keepers: harmless matmuls that bridge the PE-idle
                # window while mu/rstd/normalize run, so HAM stays at 2.4GHz
                # when the QKV projection starts
                wscr = stats_ps.tile([1, 512], F32, tag="wscr")
                for w in range(24):
                    nc.tensor.matmul(
                        wscr[:], ones_col[:], xt[w % EC][:, 0:512],
                        start=True, stop=True,
                    )

                mu_row = row_pool.tile([1, N], F32, tag="row", name="mu_row")
                nc.vector.tensor_scalar_mul(mu_row[:], ps_sum[:], 1.0 / E)
                msq_row = row_pool.tile([1, N], F32, tag="row", name="msq_row")
                nc.vector.tensor_tensor(msq_row[:], mu_row[:], mu_row[:], ALU.mult)
                var_row = row_pool.tile([1, N], F32, tag="row", name="var_row")
                nc.vector.scalar_tensor_tensor(
                    var_row[:], ps_sq[:], 1.0 / E, msq_row[:],
                    ALU.mult, ALU.subtract,
                )
            eps_ap = row_pool.tile([1, 1], F32)
            nc.vector.memset(eps_ap[:], EPS)
            # rstd = exp(-0.5 * ln(var + eps)) — both on ACT (fast row ops)
            lnv_row = row_pool.tile([1, N], F32, tag="row", name="lnv_row")
            nc.scalar.activation(lnv_row[:], var_row[:], ACTF.Ln, bias=eps_ap[:])
            rstd_row = row_pool.tile([1, N], F32, tag="row", name="rstd_row")
            nc.scalar.activation(rstd_row[:], lnv_row[:], ACTF.Exp, scale=-0.5)

            mu_b = bc_pool.tile([128, N], F32)
            nc.gpsimd.partition_broadcast(mu_b[:], mu_row[:])
            rstd_b = bc_pool.tile([128, N], F32)
            nc.gpsimd.partition_broadcast(rstd_b[:], rstd_row[:])

            # normalize: XN = (x - mu) * rstd  (LN affine folded into w_qkv).
            # Interleaved per chunk so XN[i] completes in consumption order
            # and the QKV pre-loop can start on chunk 0 immediately.
            with tc.tile_pool(name="lnt", bufs=2) as ln_pool:
                for i in range(EC):
                    tln = ln_pool.tile([128, N], F32, tag="lnt", name=f"lnt{i}")
                    nc.vector.tensor_tensor(
                        tln[:], xt[i][:], mu_b[:], ALU.subtract
                    )
                    nc.vector.tensor_tensor(XN[i][:], tln[:], rstd_b[:], ALU.mult)

        # ============ phase 2: merged QKV + attention + out-proj ============
        with (
            tc.tile_pool(name="proj_ps", bufs=1, space="PSUM") as proj_ps,
            tc.tile_pool(name="st_ps", bufs=2, space="PSUM") as st_ps,
            tc.tile_pool(name="ctx_ps", bufs=1, space="PSUM") as ctx_ps,
            tc.tile_pool(name="pt", bufs=24) as pt_pool,
            tc.tile_pool(name="stage", bufs=3) as stage_pool,
            tc.tile_pool(name="rr", bufs=2) as rr_pool,
            tc.tile_pool(name="rb", bufs=2) as rb_pool,
            tc.tile_pool(name="o_sb", bufs=2) as o_sb,
            tc.tile_pool(name="o_part", bufs=1) as o_part,
        ):
            OP = [
                o_part.tile([128, E], F16, tag=f"opart{t}", name=f"opart{t}")
                for t in range(TT)
            ]
            # ---- filler machinery: a stream of small independent PE jobs ----
            cur = {"ps": None}

            def qkt_chunk(ft, i, ps=None):
                if ps is None:
                    if i == 0:
                        cur["ps"] = proj_ps.tile(
                            [128, N], F32, tag="pps", name=f"qk{ft}"
                        )
                    ps = cur["ps"]
                for hf in range(2):
                    sl = slice(hf * 512, hf * 512 + 512)
                    nc.tensor.matmul(
                        ps[:, sl],
                        wq[i][:, ft * 128 : ft * 128 + 128],
                        XN[i][:, sl],
                        start=(i == 0), stop=(i == EC - 1),
                    )
                if i == EC - 1:
                    bias = bqk[:, ft : ft + 1]
                    dst = QT[ft] if ft < 6 else KT[ft - 6]
                    # pre-loop ftiles evacuate per half so the first score
                    # matmul isn't gated on the full-width evacuation
                    sls = (
                        (slice(0, 512), slice(512, N))
                        if ft in (0, 6, 1, 7)
                        else (slice(0, N),)
                    )
                    for sl in sls:
                        if ft < 6:
                            nc.vector.tensor_scalar(
                                dst[:, sl], ps[:, sl], bias, 1.0 / np.sqrt(HD),
                                op0=ALU.add, op1=ALU.mult,
                            )
                        else:
                            nc.vector.tensor_scalar_add(
                                dst[:, sl], ps[:, sl], bias
                            )

            def v_chunk(tt, i):
                if i == 0:
                    cur["ps"] = proj_ps.tile(
                        [128, E], F32, tag="pps", name=f"vp{tt}"
                    )
                ps = cur["ps"]
                nc.tensor.matmul(
                    ps[:, 0:512],
                    XN[i][:, tt * 128 : tt * 128 + 128],
                    wq[i][:, 1536:2048],
                    start=(i == 0), stop=(i == EC - 1),
                )
                nc.tensor.matmul(
                    ps[:, 512:768],
                    XN[i][:, tt * 128 : tt * 128 + 128],
                    wq[i][:, 2048:2304],
                    start=(i == 0), stop=(i == EC - 1),
                )
                if i == EC - 1:
                    vt = V[tt]
                    v3 = vt[:].rearrange("p (h d) -> p h d", d=65)
                    nc.vector.tensor_tensor(
                        v3[:, :, 0:64],
                        ps[:].rearrange("p (h d) -> p h d", d=64),
                        bv[:].rearrange("p (h d) -> p h d", d=64),
                        ALU.add,
                    )
                    nc.vector.tensor_copy(
                        v3[:, :, 64:65],
                        nc.const_aps.tensor(1.0, (128, 12)).unsqueeze(-1),
                    )

            def out_chunk(tt, ecs, pool=None):
                # one epoch: accumulate ec chunks `ecs` in psum, then fold
                # into the SBUF partial (or emit the final result)
                ps = (pool or proj_ps).tile(
                    [128, E], F32,
                    tag="pps" if pool is None else ("stps" if pool is st_ps else "ctxps"),
                    name=f"op{tt}_{ecs[0]}",
                )
                for j, i in enumerate(ecs):
                    nc.tensor.matmul(
                        ps[:, 0:512],
                        CT[i][:, tt * 128 : tt * 128 + 128],
                        wo[i][:, 0:512],
                        start=(j == 0), stop=(j == len(ecs) - 1),
                    )
                    nc.tensor.matmul(
                        ps[:, 512:768],
                        CT[i][:, tt * 128 : tt * 128 + 128],
                        wo[i][:, 512:768],
                        start=(j == 0), stop=(j == len(ecs) - 1),
                    )
                if ecs[0] == 0:
                    # first epoch: partial = psum + bias
                    nc.vector.tensor_tensor(OP[tt][:], ps[:], bo[:], ALU.add)
                elif ecs[-1] != EC - 1:
                    nc.vector.tensor_tensor(OP[tt][:], ps[:], OP[tt][:], ALU.add)
                else:
                    ot = o_sb.tile([128, E], F32, tag="osb", name=f"ot{tt}")
                    nc.vector.tensor_tensor(ot[:], ps[:], OP[tt][:], ALU.add)
                    nc.sync.dma_start(
                        out_d[tt * 128 : (tt + 1) * 128, :], ot[:]
                    )

            fillers = []

            def run_fillers(k):
                for _ in range(k):
                    if fillers:
                        fillers.pop(0)()

            # ---- per-head evacuate + normalize ----
            def evac_norm(h, cps):
                # stage the ctx+den PSUM to SBUF (frees the PSUM bank pair)
                stg = stage_pool.tile([65, N], F32, tag="stg", name=f"stg{h}")
                nc.vector.tensor_copy(stg[:], cps[:])
                # den row to partition 0 (cross-partition moves go via DMA)
                rr = rr_pool.tile([1, N], F32, tag="rr", name=f"rr{h}")
                nc.sync.dma_start(rr[:], stg[64:65, :])
                # 1/den (approx is plenty: ~18 correct bits)
                rri = rr_pool.tile([1, N], F32, tag="rri", name=f"rri{h}")
                nc.vector.reciprocal_approx_fast(rri[:], rr[:])
                rb = rb_pool.tile([64, N], F32, tag="rb", name=f"rb{h}")
                nc.gpsimd.partition_broadcast(rb[:], rri[:])
                pofs = (h % 2) * 64
                dest = CT[h // 2][pofs : pofs + 64, :]
                nc.vector.tensor_tensor(dest, stg[0:64, :], rb[:], ALU.mult)

            # ---- pre-loop: Q/K ftiles for pairs 0 and 1 ----
            # i-major so each chunk's matmuls run as its XN lands (the LN
            # chain produces XN[i] every ~2us; ft-major would stall the PE
            # long enough for HAM to re-throttle). Four concurrent PSUM
            # accumulators borrowed from the still-idle attention pools.
            pre_ps = {
                0: proj_ps.tile([128, N], F32, tag="pps", name="pre0"),
                6: st_ps.tile([128, N], F32, tag="stps", name="pre6"),
                1: st_ps.tile([128, N], F32, tag="stps", name="pre1"),
                7: ctx_ps.tile([128, N], F32, tag="ctxps", name="pre7"),
            }
            for i in range(EC):
                for ft in (0, 6, 1, 7):
                    qkt_chunk(ft, i, ps=pre_ps[ft])

            # V projection + remaining Q/K ftiles paced as fillers
            for tt in range(TT):
                for i in range(EC):
                    fillers.append(lambda tt=tt, i=i: v_chunk(tt, i))
            for ft in (2, 8, 3, 9, 4, 10, 5, 11):
                for i in range(EC):
                    fillers.append(lambda ft=ft, i=i: qkt_chunk(ft, i))

            # ---- main attention loop over head pairs ----
            # At pair p: row-tiled concurrent score matmuls + exp for heads
            # (2p, 2p+1); PV for pair p-1 (h0 during kt 0-3, h1 during 4-7).
            prev_pts = None  # (pts_h0, pts_h1) of previous pair
            cps_cur = None
            for p in range(NP):
                h0, h1 = 2 * p, 2 * p + 1
                pts0, pts1 = [], []
                for kt in range(TT):
                    # --- PV work for the previous pair ---
                    if prev_pts is not None:
                        ph = 2 * (p - 1) + (0 if kt < 4 else 1)
                        ppts = prev_pts[0] if kt < 4 else prev_pts[1]
                        if kt == 0:
                            cps_cur = ctx_ps.tile(
                                [65, N], F32, tag="ctxps", name=f"cps{ph}"
                            )
                        elif kt == 4:
                            evac_norm(ph - 1, cps_cur)
                            cps_cur = ctx_ps.tile(
                                [65, N], F32, tag="ctxps", name=f"cps{ph}"
                            )
                        for kk in (2 * (kt % 4), 2 * (kt % 4) + 1):
                            vch = V[kk][:, 65 * ph : 65 * ph + 65]
                            for hf in range(2):
                                sl = slice(hf * 512, hf * 512 + 512)
                                nc.tensor.matmul(
                                    cps_cur[:, sl], vch, ppts[kk][:, sl],
                                    start=(kk == 0), stop=(kk == TT - 1),
                                )
                    # --- row-tiled score pair: h0 rows 0-63, h1 rows 64-127 ---
                    psA = st_ps.tile([128, N], F32, tag="stps", name=f"stA{p}_{kt}")
                    psB = st_ps.tile([128, N], F32, tag="stps", name=f"stB{p}_{kt}")
                    ksl = slice(kt * 128, kt * 128 + 128)
                    for hf in range(2):
                        sl = slice(hf * 512, hf * 512 + 512)
                        nc.tensor.matmul(
                            psA[:, sl],
                            KT[p][0:64, ksl], QT[p][0:64, sl],
                            start=True, stop=True, tile_position=(0, 0),
                        )
                        nc.tensor.matmul(
                            psB[:, sl],
                            KT[p][64:128, ksl], QT[p][64:128, sl],
                            start=True, stop=True, tile_position=(64, 0),
                        )
                    pt0 = pt_pool.tile([128, N], F16, tag="pt", name=f"pt{h0}_{kt}")
                    nc.scalar.activation(pt0[:], psA[:], ACTF.Exp, bias=negc[:])
                    pts0.append(pt0)
                    pt1 = pt_pool.tile([128, N], F16, tag="pt", name=f"pt{h1}_{kt}")
                    nc.scalar.activation(pt1[:], psB[:], ACTF.Exp, bias=negc[:])
                    pts1.append(pt1)
                    run_fillers(6 if p == 0 else 3)
                if prev_pts is not None:
                    evac_norm(2 * p - 1, cps_cur)
                    # out-proj epochs as CT chunks complete (CT[p-1] just done)
                    if p - 1 == 1:
                        for tt in range(TT):
                            fillers.append(lambda tt=tt: out_chunk(tt, (0, 1)))
                    elif p - 1 == 3:
                        for tt in range(TT):
                            fillers.append(lambda tt=tt: out_chunk(tt, (2, 3)))
                prev_pts = (pts0, pts1)

            # ---- drain: PV + evac/normalize for the last pair ----
            # CT[4] completed at the end of the main loop; its epoch keeps
            # the PE busy (and HAM warm) while the last heads normalize.
            # PSUM slots alternate with the (idle) score pool so each tile's
            # matmuls overlap the previous tile's DVE fold.
            for tt in range(TT):
                fillers.append(
                    lambda tt=tt: out_chunk(tt, (4,), pool=st_ps if tt % 2 else None)
                )
            for ph in (H - 2, H - 1):
                ppts = prev_pts[0] if ph == H - 2 else prev_pts[1]
                # h11's ctx comes from the score-PSUM pool (idle in the
                # drain) so its PV needn't wait for h10's evacuation
                pool = ctx_ps if ph == H - 2 else st_ps
                cps_cur = pool.tile(
                    [65, N], F32,
                    tag="ctxps" if ph == H - 2 else "stps",
                    name=f"cps{ph}",
                )
                for kk in range(TT):
                    vch = V[kk][:, 65 * ph : 65 * ph + 65]
                    for hf in range(2):
                        sl = slice(hf * 512, hf * 512 + 512)
                        nc.tensor.matmul(
                            cps_cur[:, sl], vch, ppts[kk][:, sl],
                            start=(kk == 0), stop=(kk == TT - 1),
                        )
                    run_fillers(2)
                evac_norm(ph, cps_cur)
            run_fillers(len(fillers))
            # warmth keepers bridge the normalize chain of the last heads so
            # the final out-proj epoch runs at 2.4GHz
            wtail = st_ps.tile([128, 512], F32, tag="stps", name="wtail")
            for w in range(12):
                nc.tensor.matmul(
                    wtail[:], XN[w % EC][:, 0:128], wq[0][:, 0:512],
                    start=True, stop=True,
                )
            for tt in range(TT):
                out_chunk(tt, (5,), pool=ctx_ps if tt % 2 else None)


def _prep_in_maps(x, ln_g, ln_b, w_qkv, b_qkv, w_out, b_out):
    x = np.asarray(x, np.float32)
    ln_g = np.asarray(ln_g, np.float32)
    ln_b = np.asarray(ln_b, np.float32)
    w_qkv = np.asarray(w_qkv, np.float32)
    b_qkv = np.asarray(b_qkv, np.float32)
    w_out = np.asarray(w_out, np.float32)
    b_out = np.asarray(b_out, np.float32)

    # Fold the LN affine into the packed projection:
    #   qkv = (xhat*g + b) @ W^T + b_qkv = xhat @ (W*g)^T + (W @ b + b_qkv)
    w_fold = w_qkv * ln_g[None, :]
    b_fold = b_qkv + w_qkv @ ln_b

    wqkvT = np.ascontiguousarray(w_fold.T.astype(np.float16))  # [E, 3E]
    woutT = np.ascontiguousarray(w_out.T.astype(np.float16))  # [E, E]
    bqk = np.ascontiguousarray(b_fold[:1536].reshape(12, 128).T)  # [128, 12]
    bv_b = np.ascontiguousarray(np.broadcast_to(b_fold[1536:], (128, E)))
    bo_b = np.ascontiguousarray(np.broadcast_to(b_out, (128, E)))

    in_maps = []
    for c in range(B):
        in_maps.append(
            {
                "xT": np.ascontiguousarray(x[c].T.astype(np.float16)),
                "wqkvT": wqkvT,
                "woutT": woutT,
                "bqk": bqk,
                "bv_b": bv_b,
                "bo_b": bo_b,
            }
        )
    return in_maps


def run(trace=False, **inputs):
    if "nc" not in _cache:
        _cache["nc"] = _build_kernel()
    nc = _cache["nc"]
    in_maps = _prep_in_maps(**inputs)
    res = bass_utils.run_bass_kernel_spmd(
        nc, in_maps, core_ids=list(range(B)), trace=trace
    )
    out = np.stack([res.results[c]["out"] for c in range(B)], axis=0)
    return out, res


def kernel(**inputs):
    out, _ = run(trace=False, **inputs)
    return out


if __name__ == "__main__":
    rng = np.random.default_rng(0)
    inputs = {
        "x": rng.standard_normal((B, N, E), dtype=np.float32),
        "ln_g": np.ones(E, np.float32),
        "ln_b": np.zeros(E, np.float32),
        "w_qkv": rng.standard_normal((F3, E), dtype=np.float32) / np.sqrt(E),
        "b_qkv": np.zeros(F3, np.float32),
        "w_out": rng.standard_normal((E, E), dtype=np.float32) / np.sqrt(E),
        "b_out": np.zeros(E, np.float32),
    }
    y = kernel(**inputs)
    print("out shape", y.shape, "mean", float(np.abs(y).mean()))
